# revision 17
# baseline (speedup 1.0000x reference)
"""Trainium2 Bass kernel for nn_DGG_StraightThrough.

The reference's pairwise-logit MLP is mathematically dead: softmax over the
singleton feature dim is identically 1, so log_p == 0 and the gumbel logits
y equal `temp` exactly (bit-for-bit, verified).  The output is therefore the
straight-through top-8 row indicator of temp, identical for every batch
entry:  adj[b,i,j] = 1.0 iff temp[i,j] is among the 8 largest of row i.

Sharding: row-parallel over N=2048 across 8 cores (256 rows each).  Each
core: DMA its [256,2048] slab in, DVE Max8 for the per-row 8th-largest
threshold, is_ge compare to build the 0/1 mask, DMA the mask out.  Host
concatenates the 8 slabs and broadcasts over B=4.
"""

import sys

import numpy as np

if "/opt/trn_rl_repo" not in sys.path:
    sys.path.insert(0, "/opt/trn_rl_repo")

B, N, K = 4, 2048, 8
N_CORES = 8
ROWS = N // N_CORES  # 256 rows per core
P = 128  # SBUF partitions

# Hooks for a driving harness (test.py): extra kwargs for run_bass_kernel_spmd
# and the last BassKernelResults (exec_time_ns etc).
RUN_KWARGS: dict = {}
LAST_RESULT = None

_PROGRAM = None


def _build_program():
    import concourse.bass as bass
    import concourse.mybir as mybir

    class _LeanBass(bass.Bass):
        # Skip the barrier Bass.__init__ emits after const-AP registration:
        # this kernel never reads const APs, Sync's DGE table load precedes
        # its DMAs in program order, and the NRT entry pseudo-barrier already
        # orders the gpsimd sem-clears.  Saves ~1us of preamble.
        _skip_init_barrier = False

        def all_engine_barrier(self, **kw):
            if _LeanBass._skip_init_barrier:
                return
            return super().all_engine_barrier(**kw)

    _LeanBass._skip_init_barrier = True
    try:
        nc = _LeanBass(enable_partition_id=False, monotonic_sem_count=0)
    finally:
        _LeanBass._skip_init_barrier = False
    t_in = nc.declare_dram_parameter("t", [ROWS, N], mybir.dt.float32, isOutput=False)
    # u8 wire format for the 0/1 mask (lossless); host casts back to f32
    out = nc.declare_dram_parameter("out", [ROWS, N], mybir.dt.uint8, isOutput=True)

    nblk = ROWS // P  # 128-row chunks, pipelined in -> max -> cmp -> out

    H = N // 2  # column half for hierarchical MAX8
    with (
        nc.sbuf_tensor([P, nblk * N], mybir.dt.float32) as tile,
        nc.sbuf_tensor([P, nblk * N], mybir.dt.uint8) as mask,
        # per chunk: [top8 of half a | top8 of half b | merged top8]
        nc.sbuf_tensor([P, 24 * nblk], mybir.dt.float32) as top8,
        # per-transfer in-DMA sems: HWDGE transfers on different queues can
        # complete out of order, so shared counting sems would race
        nc.semaphore("in_sem0a") as in_sem0a,
        nc.semaphore("in_sem0b") as in_sem0b,
        nc.semaphore("in_sem1a") as in_sem1a,
        nc.semaphore("in_sem1b") as in_sem1b,
        nc.semaphore("out_sem") as out_sem,
        nc.semaphore("v_sem") as v_sem,
    ):
        in_sems = [[in_sem0a, in_sem0b], [in_sem1a, in_sem1b]]
        assert nblk == 2

        # Issue the in-DMAs OUTSIDE the Block, directly after Sync's DGE-table
        # preamble: they depend on no other engine, so they need not wait for
        # the block-entry all-engine sync.  Column-halves in order: the DMA
        # queue serializes them, so half a of chunk 0 lands first and MAX8
        # starts while half b is still streaming.
        for b in range(nblk):
            for h in range(2):
                nc.sync.dma_start(
                    out=tile[:, b * N + h * H : b * N + (h + 1) * H],
                    in_=t_in[b * P : (b + 1) * P, h * H : (h + 1) * H],
                ).then_inc(in_sems[b][h], 16)

        # no SWDGE DMAs issued -> skip GpSimd's expensive dge_drain at exit
        with nc.Block(no_gpsimd_drain=True) as block:

            @block.sync
            def _(sync):
                for b in range(nblk):
                    sync.wait_ge(v_sem, 4 * (b + 1))
                    sync.dma_start(
                        out=out[b * P : (b + 1) * P, :],
                        in_=mask[:, b * N : (b + 1) * N],
                    ).then_inc(out_sem, 16)
                sync.wait_ge(out_sem, 16 * nblk)

            @block.vector
            def _(vector):
                # DVE: per chunk, MAX8 each column half as it lands, merge the
                # 8+8 candidates with a third MAX8 (exact: any row-top-8
                # element is in its half's top-8), then is_ge against the 8th.
                # v_sem counts all DVE ops (in-order engine): chunk b ops are
                # 4b+1..4b+4.  The sem hops guard same-engine RAW on top8
                # (MAX8 stream-read and tensor_scalar scalar-ptr fetch race
                # the in-pipeline write of the previous op).
                for b in range(nblk):
                    t8 = top8[:, 24 * b : 24 * (b + 1)]
                    for h in range(2):
                        vector.wait_ge(in_sems[b][h], 16)
                        vector.max(
                            t8[:, 8 * h : 8 * (h + 1)],
                            tile[:, b * N + h * H : b * N + (h + 1) * H],
                        ).then_inc(v_sem, 1)
                    vector.wait_ge(v_sem, 4 * b + 2)
                    vector.max(t8[:, 16:24], t8[:, 0:16]).then_inc(v_sem, 1)
                    vector.wait_ge(v_sem, 4 * b + 3)
                    # mask = (t >= 8th largest of its row) -> 1.0 / 0.0
                    vector.tensor_scalar(
                        mask[:, b * N : (b + 1) * N],
                        tile[:, b * N : (b + 1) * N],
                        t8[:, 23:24],
                        None,
                        mybir.AluOpType.is_ge,
                    ).then_inc(v_sem, 1)
    return nc


def kernel(**inputs: np.ndarray) -> np.ndarray:
    global _PROGRAM, LAST_RESULT
    from concourse.bass_utils import run_bass_kernel_spmd

    temp = np.ascontiguousarray(np.asarray(inputs["temp"], dtype=np.float32))
    assert temp.shape == (N, N)

    if _PROGRAM is None:
        _PROGRAM = _build_program()

    in_maps = [
        {"t": np.ascontiguousarray(temp[c * ROWS : (c + 1) * ROWS])}
        for c in range(N_CORES)
    ]
    res = run_bass_kernel_spmd(_PROGRAM, in_maps, list(range(N_CORES)), **RUN_KWARGS)
    LAST_RESULT = res

    mask = np.concatenate([res.results[c]["out"] for c in range(N_CORES)], axis=0)
    mask = mask.astype(np.float32)
    return np.ascontiguousarray(np.broadcast_to(mask[None], (B, N, N)))


# revision 18
# speedup vs baseline: 1.0086x; 1.0086x over previous
"""Trainium2 Bass kernel for nn_DGG_StraightThrough.

The reference's pairwise-logit MLP is mathematically dead: softmax over the
singleton feature dim is identically 1, so log_p == 0 and the gumbel logits
y equal `temp` exactly (bit-for-bit, verified).  The output is therefore the
straight-through top-8 row indicator of temp, identical for every batch
entry:  adj[b,i,j] = 1.0 iff temp[i,j] is among the 8 largest of row i.

Sharding: row-parallel over N=2048 across 8 cores (256 rows each).  Each
core: DMA its [256,2048] slab in, DVE Max8 for the per-row 8th-largest
threshold, is_ge compare to build the 0/1 mask, DMA the mask out.  Host
concatenates the 8 slabs and broadcasts over B=4.
"""

import sys

import numpy as np

if "/opt/trn_rl_repo" not in sys.path:
    sys.path.insert(0, "/opt/trn_rl_repo")

B, N, K = 4, 2048, 8
N_CORES = 8
ROWS = N // N_CORES  # 256 rows per core
P = 128  # SBUF partitions

# Hooks for a driving harness (test.py): extra kwargs for run_bass_kernel_spmd
# and the last BassKernelResults (exec_time_ns etc).
RUN_KWARGS: dict = {}
LAST_RESULT = None

_PROGRAM = None


def _build_program():
    import concourse.bass as bass
    import concourse.mybir as mybir

    class _LeanBass(bass.Bass):
        # Skip the barrier Bass.__init__ emits after const-AP registration:
        # this kernel never reads const APs, Sync's DGE table load precedes
        # its DMAs in program order, and the NRT entry pseudo-barrier already
        # orders the gpsimd sem-clears.  Saves ~1us of preamble.
        _skip_init_barrier = False

        def all_engine_barrier(self, **kw):
            if _LeanBass._skip_init_barrier:
                return
            return super().all_engine_barrier(**kw)

    _LeanBass._skip_init_barrier = True
    try:
        nc = _LeanBass(enable_partition_id=False, monotonic_sem_count=0)
    finally:
        _LeanBass._skip_init_barrier = False
    t_in = nc.declare_dram_parameter("t", [ROWS, N], mybir.dt.float32, isOutput=False)
    # u8 wire format for the 0/1 mask (lossless); host casts back to f32
    out = nc.declare_dram_parameter("out", [ROWS, N], mybir.dt.uint8, isOutput=True)

    nblk = ROWS // P  # 128-row chunks, pipelined in -> max -> cmp -> out

    H = N // 2  # column half for hierarchical MAX8
    with (
        nc.sbuf_tensor([P, nblk * N], mybir.dt.float32) as tile,
        nc.sbuf_tensor([P, nblk * N], mybir.dt.uint8) as mask,
        # per chunk: [top8 of half a | top8 of half b | merged top8]
        nc.sbuf_tensor([P, 24 * nblk], mybir.dt.float32) as top8,
        # per-transfer in-DMA sems: HWDGE transfers on different queues can
        # complete out of order, so shared counting sems would race
        nc.semaphore("in_sem0a") as in_sem0a,
        nc.semaphore("in_sem0b") as in_sem0b,
        nc.semaphore("in_sem1") as in_sem1,
        nc.semaphore("out_sem") as out_sem,
        nc.semaphore("v_sem") as v_sem,
    ):
        assert nblk == 2

        # Issue the in-DMAs OUTSIDE the Block, directly after Sync's DGE-table
        # preamble: they depend on no other engine, so they need not wait for
        # the block-entry all-engine sync.  Chunk 0 arrives as two column
        # halves so its first MAX8 starts while the rest still streams; chunk 1
        # is one transfer (it is DVE-gated anyway, so splitting only adds
        # merge overhead).
        nc.sync.dma_start(out=tile[:, 0:H], in_=t_in[0:P, 0:H]).then_inc(in_sem0a, 16)
        nc.sync.dma_start(out=tile[:, H:N], in_=t_in[0:P, H:N]).then_inc(in_sem0b, 16)
        nc.sync.dma_start(out=tile[:, N : 2 * N], in_=t_in[P : 2 * P, :]).then_inc(
            in_sem1, 16
        )

        # no SWDGE DMAs issued -> skip GpSimd's expensive dge_drain at exit
        with nc.Block(no_gpsimd_drain=True) as block:

            @block.sync
            def _(sync):
                for b, v_target in enumerate((4, 6)):
                    sync.wait_ge(v_sem, v_target)
                    sync.dma_start(
                        out=out[b * P : (b + 1) * P, :],
                        in_=mask[:, b * N : (b + 1) * N],
                    ).then_inc(out_sem, 16)
                sync.wait_ge(out_sem, 16 * nblk)

            @block.vector
            def _(vector):
                # Chunk 0: MAX8 each column half as it lands, merge the 8+8
                # candidates with a third MAX8 (exact: any row-top-8 element is
                # in its half's top-8), then is_ge against the merged 8th.
                # Chunk 1: flat MAX8 + is_ge.  v_sem counts all DVE ops
                # (in-order engine).  The sem hops guard same-engine RAW on
                # top8 (MAX8 stream-read and tensor_scalar scalar-ptr fetch
                # race the in-pipeline write of the previous op).
                vector.wait_ge(in_sem0a, 16)
                vector.max(top8[:, 0:8], tile[:, 0:H]).then_inc(v_sem, 1)
                vector.wait_ge(in_sem0b, 16)
                vector.max(top8[:, 8:16], tile[:, H:N]).then_inc(v_sem, 1)
                vector.wait_ge(v_sem, 2)
                vector.max(top8[:, 16:24], top8[:, 0:16]).then_inc(v_sem, 1)
                vector.wait_ge(v_sem, 3)
                # mask = (t >= 8th largest of its row) -> 1.0 / 0.0
                vector.tensor_scalar(
                    mask[:, 0:N],
                    tile[:, 0:N],
                    top8[:, 23:24],
                    None,
                    mybir.AluOpType.is_ge,
                ).then_inc(v_sem, 1)
                vector.wait_ge(in_sem1, 16)
                vector.max(top8[:, 24:32], tile[:, N : 2 * N]).then_inc(v_sem, 1)
                vector.wait_ge(v_sem, 5)
                vector.tensor_scalar(
                    mask[:, N : 2 * N],
                    tile[:, N : 2 * N],
                    top8[:, 31:32],
                    None,
                    mybir.AluOpType.is_ge,
                ).then_inc(v_sem, 1)
    return nc


def kernel(**inputs: np.ndarray) -> np.ndarray:
    global _PROGRAM, LAST_RESULT
    from concourse.bass_utils import run_bass_kernel_spmd

    temp = np.ascontiguousarray(np.asarray(inputs["temp"], dtype=np.float32))
    assert temp.shape == (N, N)

    if _PROGRAM is None:
        _PROGRAM = _build_program()

    in_maps = [
        {"t": np.ascontiguousarray(temp[c * ROWS : (c + 1) * ROWS])}
        for c in range(N_CORES)
    ]
    res = run_bass_kernel_spmd(_PROGRAM, in_maps, list(range(N_CORES)), **RUN_KWARGS)
    LAST_RESULT = res

    mask = np.concatenate([res.results[c]["out"] for c in range(N_CORES)], axis=0)
    mask = mask.astype(np.float32)
    return np.ascontiguousarray(np.broadcast_to(mask[None], (B, N, N)))
